# revision 1
# baseline (speedup 1.0000x reference)
"""NetVLAD layer kernel for Trainium2 (Bass/Tile), data-parallel over batch on 8 cores.

Math (per image):
  s = x @ Wk + bias          # [HW, K]   x:[HW, D], Wk:[D, K]
  a = softmax(s, axis=-1)    # [HW, K]
  vT[k, d] = sum_p a[p,k] x[p,d] + (sum_p a[p,k]) * C[d,k]
  intra L2-normalize over d -> global L2-normalize -> out [K*D]

Sharding: batch 32 -> 4 images per core; Wk/bias/C replicated.
Precision: x and Wk cast to bf16 on host for the matmuls (PSUM accumulates
fp32); softmax and normalization arithmetic are fp32.
"""

import sys

sys.path.insert(0, "/opt/trn_rl_repo")

import numpy as np
import ml_dtypes

import concourse.bacc as bacc
import concourse.bass as bass
import concourse.mybir as mybir
import concourse.tile as tile
from concourse import masks
from concourse import bass_utils

F32 = mybir.dt.float32
BF16 = mybir.dt.bfloat16

N_CORES = 8
B = 32
H, W_IMG, D, K = 60, 80, 512, 64
HW = H * W_IMG            # 4800 pixels per image
B_LOC = B // N_CORES      # 4 images per core
P = 128                   # partition / pixel-chunk size
NDC = D // P              # 4 D-chunks
CHUNKS = [(i * P, P) for i in range(HW // P)] + (
    [(HW - HW % P, HW % P)] if HW % P else []
)
NCH = len(CHUNKS)

EPS = 1e-12


class _patched_act_tables:
    """Context manager: force the act-table-load pass to use the one set that
    contains Exp, Ln and Copy, so the kernel never swaps ACT tables. Restores
    the original lookup on exit (it is global concourse state)."""

    def __enter__(self):
        from concourse import hw_specs
        import functools

        self._orig_hw = hw_specs.get_activation_tables
        self._orig_bacc = bacc.get_activation_tables

        orig = self._orig_hw

        @functools.cache
        def patched(arch):
            tabs = dict(orig(arch))
            if "natural_log_exp_and_others" in tabs:
                tabs = {
                    name: (s if name == "natural_log_exp_and_others" else set())
                    for name, s in tabs.items()
                }
            return tabs

        hw_specs.get_activation_tables = patched
        bacc.get_activation_tables = patched

    def __exit__(self, *exc):
        from concourse import hw_specs

        hw_specs.get_activation_tables = self._orig_hw
        bacc.get_activation_tables = self._orig_bacc
        return False


def build_netvlad(reps: int = 1):
    """Build and compile the per-core Bass program. reps>1 wraps the whole
    computation in a hardware For loop (for timing measurements)."""
    with _patched_act_tables():
        return _build_netvlad_inner(reps)


def _build_netvlad_inner(reps: int):
    nc = bacc.Bacc("TRN2", target_bir_lowering=False, debug=False, num_devices=N_CORES)

    x_d = nc.dram_tensor("x", [B_LOC, HW, D], BF16, kind="ExternalInput").ap()
    wk_d = nc.dram_tensor("wk", [D, K], BF16, kind="ExternalInput").ap()
    # bias split into bf16 hi+lo rows; added to s via a rank-2 matmul
    bias_d = nc.dram_tensor("bias2", [2, K], BF16, kind="ExternalInput").ap()
    ct_d = nc.dram_tensor("ct", [K, D], F32, kind="ExternalInput").ap()
    out_d = nc.dram_tensor("out", [B_LOC, K * D], F32, kind="ExternalOutput").ap()

    mult = mybir.AluOpType.mult
    add = mybir.AluOpType.add
    AF = mybir.ActivationFunctionType

    with tile.TileContext(nc) as tc:
        from contextlib import ExitStack

        with ExitStack() as ctx:
            singles = ctx.enter_context(tc.tile_pool(name="singles", bufs=1))
            xin = ctx.enter_context(tc.tile_pool(name="xin", bufs=6))
            xtp = ctx.enter_context(tc.tile_pool(name="xtp", bufs=3))
            soft = ctx.enter_context(tc.tile_pool(name="soft", bufs=4))
            fin = ctx.enter_context(tc.tile_pool(name="fin", bufs=2))
            pt = ctx.enter_context(tc.tile_pool(name="pt", bufs=3, space="PSUM"))
            ps = ctx.enter_context(tc.tile_pool(name="ps", bufs=2, space="PSUM"))
            pv = ctx.enter_context(tc.tile_pool(name="pv", bufs=2, space="PSUM"))
            pa = ctx.enter_context(tc.tile_pool(name="pa", bufs=1, space="PSUM"))

            # ---- constants (loaded once) ----
            wk_sb = singles.tile([P, NDC, K], BF16)  # [d_in_chunk, c, k]
            nc.gpsimd.dma_start(out=wk_sb, in_=wk_d.rearrange("(c p) k -> p c k", p=P))
            bias_sb = singles.tile([2, K], BF16)
            nc.gpsimd.dma_start(out=bias_sb, in_=bias_d)
            ones2 = singles.tile([2, P], BF16)
            nc.vector.memset(ones2[:], 1.0)
            ct_sb = singles.tile([K, D], F32)
            nc.gpsimd.dma_start(out=ct_sb, in_=ct_d)
            ident = singles.tile([P, P], BF16)
            masks.make_identity(nc, ident[:])
            ones_col = singles.tile([P, 1], BF16)
            nc.vector.memset(ones_col[:], 1.0)
            ones_col_f = singles.tile([P, 1], F32)
            nc.vector.memset(ones_col_f[:], 1.0)
            ones_row_f = singles.tile([1, K], F32)
            nc.vector.memset(ones_row_f[:], 1.0)
            eps_sb = singles.tile([K, 1], F32)
            nc.vector.memset(eps_sb[:], EPS)

            def emit_front(b, ci, state, supers):
                p0, psz = CHUNKS[ci]
                # x loaded in 512-pixel super-chunks: one DMA per 4 chunks
                # (HWDGE per-DMA overhead is the serial resource)
                if ci % 4 == 0:
                    xsup = xin.tile([P, 4, D], BF16, tag="x", name="xsup")
                    if ci + 4 <= NCH - 2:  # full 512-pixel super-chunk
                        nc.sync.dma_start(
                            out=xsup[:],
                            in_=x_d[b, p0 : p0 + 4 * P, :].rearrange(
                                "(q p) d -> p q d", p=P
                            ),
                        )
                    else:  # tail super-chunk: remaining chunks, one DMA each
                        for cj in range(ci, NCH):
                            pj, pjsz = CHUNKS[cj]
                            nc.sync.dma_start(
                                out=xsup[:pjsz, cj - ci, :],
                                in_=x_d[b, pj : pj + pjsz, :],
                            )
                    supers["cur"] = xsup
                x_sb = supers["cur"][:, ci % 4, :]
                xt_sb = xtp.tile([P, NDC, P], BF16, tag="xt", name="xt_sb")
                ptile = pt.tile([P, NDC, P], BF16, tag="pt", name="ptile")
                for c in range(NDC):
                    nc.tensor.transpose(
                        ptile[:P, c, :psz],
                        x_sb[:psz, c * P : (c + 1) * P],
                        ident[:psz, :psz],
                    )
                # evacuate PSUM->SBUF as packed int32 (2 bf16/element) on DVE
                if psz == P:
                    nc.vector.tensor_copy(
                        out=xt_sb.bitcast(mybir.dt.int32),
                        in_=ptile.bitcast(mybir.dt.int32),
                    )
                else:
                    for c in range(NDC):
                        nc.vector.tensor_copy(
                            out=xt_sb[:, c, :psz].bitcast(mybir.dt.int32),
                            in_=ptile[:, c, :psz].bitcast(mybir.dt.int32),
                        )
                s_ps = ps.tile([P, K], F32, tag="s", name="s_ps")
                for c in range(NDC):
                    nc.tensor.matmul(
                        s_ps[:psz],
                        xt_sb[:, c, :psz],
                        wk_sb[:, c, :],
                        start=(c == 0),
                        stop=False,
                    )
                # bias add fused into the accumulation group (hi+lo rows)
                nc.tensor.matmul(
                    s_ps[:psz], ones2[:, :psz], bias_sb[:], start=False, stop=True
                )
                state[ci] = (x_sb, s_ps, psz)

            def emit_softmax(ci, state):
                x_sb, s_ps, psz = state[ci]
                e_sb = soft.tile([P, K], F32, tag="e", name="e_sb")
                r_sb = soft.tile([P, 1], F32, tag="r", name="r_sb")
                nc.scalar.activation(
                    e_sb[:psz], s_ps[:psz], AF.Exp, accum_out=r_sb[:psz]
                )
                rinv = soft.tile([P, 1], F32, tag="rinv", name="rinv")
                nc.vector.reciprocal(rinv[:psz], r_sb[:psz])
                a_sb = soft.tile([P, K], BF16, tag="a", name="a_sb")
                nc.vector.tensor_scalar_mul(a_sb[:psz], e_sb[:psz], rinv[:psz])
                state[ci] = (x_sb, a_sb, psz)

            def emit_v(ci, j, vt, pa_t, state):
                x_sb, a_sb, psz = state.pop(ci)
                nc.tensor.matmul(
                    vt[:],
                    a_sb[:psz],
                    x_sb[:psz],
                    start=(ci == 0),
                    stop=(ci == NCH - 1),
                )
                # both images' asum columns share one PSUM bank / one
                # accumulation group (start on first write, stop on last)
                nc.tensor.matmul(
                    pa_t[:, j : j + 1],
                    a_sb[:psz],
                    ones_col[:psz],
                    start=(ci == 0 and j == 0),
                    stop=(ci == NCH - 1 and j == 1),
                )

            def body():
                for pair in range(B_LOC // 2):
                    bs = [2 * pair, 2 * pair + 1]
                    # per-image PSUM accumulators (one bank each):
                    # vt: [K, D] sums; pa col 0: asum; col 1: g; col 2: g bcast
                    vts = [pv.tile([K, D], F32, tag="vt_ps", name="vt_ps") for _ in bs]
                    # shared pair accumulator bank: cols 0-1 asum(img0,img1),
                    # cols 2-3 g, cols 4-5 g-broadcast
                    pa_t = pa.tile([K, 6], F32, tag="pa_t", name="pa_t")
                    states = [{}, {}]
                    supers = [{}, {}]
                    # two independent image streams interleaved; per stream a
                    # 3-stage software pipeline (front / softmax-1 / v-2)
                    for ci in range(NCH + 2):
                        for j in range(2):
                            if ci < NCH:
                                emit_front(bs[j], ci, states[j], supers[j])
                        for j in range(2):
                            if 0 <= ci - 1 < NCH:
                                emit_softmax(ci - 1, states[j])
                        for j in range(2):
                            if ci >= 2:
                                emit_v(ci - 2, j, vts[j], pa_t, states[j])
                    for j in range(2):
                        finalize(bs[j], j, vts[j], pa_t)

            def finalize(b, j, vt_ps, pa_t):
                    # ---- finalize image b ----
                    asum_sb = fin.tile([K, 1], F32, tag="asum_sb")
                    nc.scalar.copy(out=asum_sb[:], in_=pa_t[:, j : j + 1])
                    # vT[k, d] = vt_ps + asum[k] * C[d, k]
                    vt_sb = fin.tile([K, D], F32, tag="vt")
                    nc.vector.scalar_tensor_tensor(
                        out=vt_sb[:],
                        in0=ct_sb[:],
                        scalar=asum_sb[:],
                        in1=vt_ps[:],
                        op0=mult,
                        op1=add,
                    )
                    # intra-norm: nsq[k] = sum_d vT[k,d]^2
                    sq_sb = fin.tile([K, D], F32, tag="sq")
                    nsq = fin.tile([K, 1], F32, tag="nsq")
                    nc.vector.tensor_mul(sq_sb[:], vt_sb[:], vt_sb[:])
                    nc.vector.reduce_sum(nsq[:], sq_sb[:], axis=mybir.AxisListType.X)
                    # rnorm = 1/sqrt(nsq+eps) = exp(-0.5*ln(nsq+eps))
                    lnn = fin.tile([K, 1], F32, tag="lnn")
                    nc.scalar.activation(lnn[:], nsq[:], AF.Ln, bias=eps_sb[:])
                    rnorm = fin.tile([K, 1], F32, tag="rnorm")
                    nc.scalar.activation(rnorm[:], lnn[:], AF.Exp, scale=-0.5)
                    # srow = nsq * rnorm^2  (post-intra-norm row energy)
                    srow = fin.tile([K, 1], F32, tag="srow")
                    nc.vector.scalar_tensor_tensor(
                        out=srow[:], in0=rnorm[:], scalar=nsq[:], in1=rnorm[:],
                        op0=mult, op1=mult,
                    )
                    # g = sum_k srow -> pa_t[0,1]; broadcast to [K,1] -> pa_t[:,2]
                    nc.tensor.matmul(
                        pa_t[0:1, 2 + j : 3 + j], srow[:], ones_col_f[:K],
                        start=True, stop=True,
                    )
                    g_sb = fin.tile([1, 1], F32, tag="g_sb")
                    nc.scalar.copy(out=g_sb[:], in_=pa_t[0:1, 2 + j : 3 + j])
                    nc.tensor.matmul(
                        pa_t[:, 4 + j : 5 + j], ones_row_f[:], g_sb[:],
                        start=True, stop=True,
                    )
                    lng = fin.tile([K, 1], F32, tag="lng")
                    nc.scalar.activation(
                        lng[:], pa_t[:, 4 + j : 5 + j], AF.Ln, bias=eps_sb[:]
                    )
                    ginv = fin.tile([K, 1], F32, tag="ginv")
                    nc.scalar.activation(ginv[:], lng[:], AF.Exp, scale=-0.5)
                    scl = fin.tile([K, 1], F32, tag="scl")
                    nc.vector.tensor_mul(scl[:], rnorm[:], ginv[:])
                    o_sb = fin.tile([K, D], F32, tag="o")
                    nc.vector.tensor_scalar_mul(o_sb[:], vt_sb[:], scl[:])
                    nc.gpsimd.dma_start(
                        out=out_d[b].rearrange("(k d) -> k d", d=D), in_=o_sb[:]
                    )

            if reps == 1:
                body()
            else:
                with tc.For_i(0, reps, 1):
                    body()

    nc.compile()
    return nc


_NC_CACHE = {}


def _get_nc(reps: int = 1):
    if reps not in _NC_CACHE:
        _NC_CACHE[reps] = build_netvlad(reps)
    return _NC_CACHE[reps]


def _make_in_maps(x, kernel, bias, C):
    wk = np.ascontiguousarray(kernel.reshape(D, K)).astype(ml_dtypes.bfloat16)
    bias_f = np.asarray(bias, dtype=np.float32).reshape(K)
    b_hi = bias_f.astype(ml_dtypes.bfloat16)
    b_lo = (bias_f - b_hi.astype(np.float32)).astype(ml_dtypes.bfloat16)
    bias2 = np.ascontiguousarray(np.stack([b_hi, b_lo], axis=0))
    ct = np.ascontiguousarray(C.reshape(D, K).T, dtype=np.float32)
    xb = np.asarray(x).astype(ml_dtypes.bfloat16)
    in_maps = []
    for i in range(N_CORES):
        xs = np.ascontiguousarray(
            xb[i * B_LOC : (i + 1) * B_LOC].reshape(B_LOC, HW, D)
        )
        in_maps.append({"x": xs, "wk": wk, "bias2": bias2, "ct": ct})
    return in_maps


def kernel(x, kernel, bias, C):
    """Full-input entry point: x [32,60,80,512], kernel [1,1,512,64],
    bias [1,1,64], C [1,1,1,512,64] -> out [32, 32768] (float32)."""
    nc = _get_nc(reps=1)
    in_maps = _make_in_maps(x, kernel, bias, C)
    res = bass_utils.run_bass_kernel_spmd(nc, in_maps, list(range(N_CORES)))
    out = np.concatenate([res.results[i]["out"] for i in range(N_CORES)], axis=0)
    return out

